# revision 60
# baseline (speedup 1.0000x reference)
"""Trainium2 Bass kernel for masked cosine-similarity attention scores.

Problem: nn_MultiHeadedAttention_2 (sparse_attention, memory-bound)
  query [16, 1, 1024] f32, key [16, 8192, 1024] f32, mask [16, 8192] int32
  out   [16, 16, 8192] f32 = relu(cos_sim_per_head(q, k) masked) / Lk

Math (per batch b, head h, key position l):
  num[h,l] = sum_d q[h,d] * k[l, h*64+d]
  s2[h,l]  = sum_d k[l, h*64+d]^2          (= kn^2)
  p        = relu(num_tilde) * exp(-0.5*ln(s2 + 1e-8))
  where num_tilde folds q/(qn*Lk) into the weights, and the host zeroes
  the masked key columns so num==0 and s2==0 there: relu(0)*exp(9.2)=0
  gives exact masked zeros. The reference's EPS=1e-8 guard on qn*kn is
  unreachable for randn inputs, so it is not emulated.

Device algorithm (v2 - TensorEngine reduction):
  The host supplies the key tensor cast to bf16 and TRANSPOSED to
  [B, d_model, Lk] with masked columns zeroed. On-chip, d_model lives on
  the partition axis (8 chunks of 128), so both per-head segmented sums
  become PE matmuls contracting over partitions:
    num[16, 512] = sum_cc qs_cc[128,16].T @ kt_cc[128,512]     (8 MMs)
    s2 [16, 512] = sum_cc oh_cc[128,16].T @ Square(kt)_cc      (8 MMs)
  qs_cc holds qtilde = q/(qn*Lk) scattered per chunk (zero off-head),
  oh_cc is the 0/1 head-membership indicator. ACT does Square + Ln/Exp,
  DVE does one relu*mult STT per 512-key tile, GPSIMD only issues the
  small output stores. This removes the v1 DVE/GPSIMD fold trees that
  dominated the measured 434us span (DVE 86% busy).

Sharding: data-parallel over batch B=16 -> 2 batches per core x 8 cores.

Self-contained: only imports the platform libs from /opt/trn_rl_repo.
"""

import sys

sys.path.insert(0, "/opt/trn_rl_repo")

import numpy as np

import concourse.bass as bass
import concourse.mybir as mybir
from concourse.tile import TileContext

# Keep the number of active DMA completion-sem lanes low: the kernel-tail
# Drain waits on every active proc's semaphore and walrus rejects
# instructions with too many sync waits.
import concourse.tile_sem_assignment as _tsa

_tsa.NUM_HWDGE_SEMS = 2
_tsa.NUM_SWDGE_GLOBAL_SEMS = 2

# The walrus build in this environment accepts at most ONE sync wait per
# instruction. Tile's scheduler can emit several (cross-engine RAW + WAR +
# DMA-lane waits). Splitting the extra waits into standalone EventSemaphore
# instructions on the same engine is semantically identical: the engine's
# sequencer executes them in order immediately before the instruction.
import orjson as _orjson


def _split_multi_waits(bir_bytes: bytes) -> bytes:
    m = _orjson.loads(bir_bytes)
    changed = False
    for fn in m.get("functions", []):
        for bb in fn.get("blocks", []):
            insts = bb.get("instructions")
            if not insts:
                continue
            out_list = []
            for inst in insts:
                si = inst.get("sync_info")
                waits = (si or {}).get("on_wait") or []
                if len(waits) > 1:
                    changed = True
                    for k, w in enumerate(waits[:-1]):
                        out_list.append(
                            {
                                "debug": inst.get("debug", 0),
                                "engine": inst["engine"],
                                "ins": [],
                                "name": f"{inst['name']}_wsplit{k}",
                                "opcode": "EventSemaphore",
                                "outs": [],
                                "sync_info": {"on_update": [], "on_wait": [w]},
                            }
                        )
                    si["on_wait"] = [waits[-1]]
                out_list.append(inst)
            bb["instructions"] = out_list
    return _orjson.dumps(m) if changed else bir_bytes


_orig_to_json_bytes = bass.Bass.to_json_bytes


def _patched_to_json_bytes(self, *a, **kw):
    return _split_multi_waits(_orig_to_json_bytes(self, *a, **kw))


bass.Bass.to_json_bytes = _patched_to_json_bytes

F32 = mybir.dt.float32
BF16 = mybir.dt.bfloat16
I32 = mybir.dt.int32
Alu = mybir.AluOpType
Act = mybir.ActivationFunctionType
AX = mybir.AxisListType

H = 16      # heads
DK = 64     # head dim
DM = 1024   # d_model
NCH = DM // 128  # d-chunks of 128 partitions
P = 128     # SBUF partitions
N_CORES = 8
NMM = 512   # matmul moving free size (= PSUM bank capacity in f32)
EPS_S2 = 1e-8


def _block_plan(n_batch: int, lk: int):
    """Per-batch (start, size) block lists: 512-key blocks at the very
    start (short pipeline ramp) and very end (short drain tail), 1024 in
    steady state (fine granularity pipelines best with double-buffered
    num-psum and deep kt prefetch)."""
    plans = []
    for b in range(n_batch):
        if lk <= 1024:
            sizes = [512] * (lk // 512) or [lk]
        else:
            n_big = (lk - 2048) // 2048
            if b == 0:
                sizes = [512, 512, 1024] + [2048] * n_big
            elif b == n_batch - 1:
                sizes = [2048] * n_big + [1024, 512, 512]
            else:
                sizes = [2048] * (n_big + 1)
        blocks, s = [], 0
        for sz in sizes:
            blocks.append((s, sz))
            s += sz
        assert s == lk, (s, lk, sizes)
        plans.append(blocks)
    return plans


def build_nc(n_batch: int, lk: int) -> bass.Bass:
    """Build the per-core Bass program.

    Per-core DRAM I/O:
      ktb [n_batch, 1024, lk] bf16  (transposed, masked-zeroed key shard)
      qs  [128, n_batch*8*16] bf16  (per-chunk qtilde stationaries)
      ohd [128, 4*2*16] fp8e4       (chunk-pair head one-hots, DoubleRow)
      out [n_batch, 16, lk] f32
    """
    plans = _block_plan(n_batch, lk)

    nc = bass.Bass()
    ktb_in = nc.declare_dram_parameter("ktb", [n_batch, DM, lk], BF16,
                                       isOutput=False)
    qs_in = nc.declare_dram_parameter("qs", [P, n_batch * NCH * H], BF16,
                                      isOutput=False)
    oh_in = nc.declare_dram_parameter("oh", [P, NCH * H], BF16,
                                      isOutput=False)
    out = nc.declare_dram_parameter("out", [n_batch, H, lk], F32,
                                    isOutput=True)

    with TileContext(nc) as tc:
        with (
            tc.tile_pool(name="const", bufs=1) as cpool,
            tc.tile_pool(name="kbig", bufs=3) as kpool,
            tc.tile_pool(name="sqp", bufs=2) as qpool,
            tc.tile_pool(name="small", bufs=4) as spool,
            tc.tile_pool(name="outp", bufs=2) as opool,
            tc.tile_pool(name="psumn", bufs=2, space="PSUM") as pnpool,
            tc.tile_pool(name="psums", bufs=3, space="PSUM") as pspool,
        ):
            # consts go on the ACT HWDGE ring (idle at start; carries only
            # the small deferred output stores later) so both key-load rings
            # start streaming immediately
            qs = cpool.tile([P, n_batch * NCH * H], BF16, name="qs")
            nc.scalar.dma_start(out=qs[:], in_=qs_in[:])
            oh = cpool.tile([P, NCH * H], BF16, name="oh")
            nc.scalar.dma_start(out=oh[:], in_=oh_in[:])
            eps = cpool.tile([H, 1], F32, name="eps")
            nc.gpsimd.memset(eps[:], EPS_S2)

            NH = NCH // 2
            pending_out = None
            for b in range(n_batch):
                ktb_b = ktb_in[b].rearrange("(cc p) l -> p cc l", p=P)
                for start, KB in plans[b]:
                    MMW = NMM  # matmul output must stay within one PSUM bank
                    nhalf = KB // MMW
                    # two half-tiles (chunks 0-3 / 4-7), one per load ring:
                    # two transfers concurrently in flight on independent
                    # rings (SP HWDGE / gpsimd SWDGE) measured fastest
                    ktA = kpool.tile([P, NH * KB], BF16, name="ktA", tag="ktA")
                    ktB = kpool.tile([P, NH * KB], BF16, name="ktB", tag="ktB")
                    ktA3 = ktA.rearrange("p (cc l) -> p cc l", l=KB)
                    ktB3 = ktB.rearrange("p (cc l) -> p cc l", l=KB)
                    nc.sync.dma_start(
                        out=ktA3, in_=ktb_b[:, 0:NH, start : start + KB]
                    )
                    nc.gpsimd.dma_start(
                        out=ktB3, in_=ktb_b[:, NH:NCH, start : start + KB]
                    )

                    def kt_cc(cc):
                        return ktA3[:, cc] if cc < NH else ktB3[:, cc - NH]

                    if pending_out is not None:
                        ppout, pb, pstart, pkb = pending_out
                        nc.scalar.dma_start(
                            out=out[pb][:, pstart : pstart + pkb], in_=ppout
                        )
                        pending_out = None

                    # k^2 split across DVE(6)/ACT(2) to balance engine load
                    sqA = qpool.tile([P, NH * KB], BF16, name="sqA", tag="sqA")
                    sqB = qpool.tile([P, NH * KB], BF16, name="sqB", tag="sqB")
                    sqA3 = sqA.rearrange("p (cc l) -> p cc l", l=KB)
                    sqB3 = sqB.rearrange("p (cc l) -> p cc l", l=KB)

                    def sq_cc(cc):
                        return sqA3[:, cc] if cc < NH else sqB3[:, cc - NH]

                    for cc in range(NCH):
                        if cc < 6:
                            nc.vector.tensor_tensor(
                                sq_cc(cc), kt_cc(cc), kt_cc(cc), Alu.mult
                            )
                        else:
                            nc.scalar.activation(sq_cc(cc), kt_cc(cc),
                                                 Act.Square)

                    pout = opool.tile([H, KB], F32, name="pout", tag="pout")
                    # stagger psum partition bases (0 / 64 for the two num
                    # halves, 32 for s2): the matmul col-group follows the
                    # output partition base, so stationaries land in
                    # different PE col-groups and their LDWEIGHTS can be
                    # pulled ahead of in-flight matmuls of the other group
                    pnums = []
                    for h in range(nhalf):
                        off = 64 * (h % 2)
                        t = pnpool.tile([off + H, MMW], F32, name="pnum",
                                        tag=f"pnum{h % 2}")
                        pnums.append(t[off : off + H])
                    # cc-outer: one LDWEIGHTS per chunk feeds all key halves
                    for cc in range(NCH):
                        for h in range(nhalf):
                            nc.tensor.matmul(
                                pnums[h][:],
                                qs[:, (b * NCH + cc) * H : (b * NCH + cc + 1) * H],
                                kt_cc(cc)[:, h * MMW : (h + 1) * MMW],
                                start=(cc == 0),
                                stop=(cc == NCH - 1),
                            )
                    for h in range(nhalf):
                        ksl = slice(h * MMW, (h + 1) * MMW)
                        ps2t = pspool.tile([32 + H, MMW], F32, name="ps2",
                                           tag="ps2")
                        ps2 = ps2t[32 : 32 + H]
                        for cc in range(NCH):
                            nc.tensor.matmul(
                                ps2[:],
                                oh[:, cc * H : (cc + 1) * H],
                                sq_cc(cc)[:, ksl],
                                start=(cc == 0),
                                stop=(cc == NCH - 1),
                            )
                        lns = spool.tile([H, MMW], F32, name="lns", tag="lns")
                        nc.scalar.activation(lns[:], ps2[:], Act.Ln,
                                             bias=eps[:])
                        rk = spool.tile([H, MMW], F32, name="rk", tag="rk")
                        nc.scalar.activation(rk[:], lns[:], Act.Exp,
                                             scale=-0.5)
                        # pout[:, ksl] = max(pnum, 0) * rk
                        nc.vector.scalar_tensor_tensor(
                            pout[:, ksl], pnums[h][:], 0.0, rk[:],
                            Alu.max, Alu.mult,
                        )
                    pending_out = (pout[:], b, start, KB)
            ppout, pb, pstart, pkb = pending_out
            nc.scalar.dma_start(out=out[pb][:, pstart : pstart + pkb],
                                in_=ppout)
    return nc


_NC_CACHE: dict = {}


def _get_nc(n_batch, lk):
    key = (n_batch, lk)
    if key not in _NC_CACHE:
        _NC_CACHE[key] = build_nc(n_batch, lk)
    return _NC_CACHE[key]


def _round_to_bf16_bits(x_f32: np.ndarray) -> np.ndarray:
    """f32 ndarray -> uint16 ndarray of bf16 bit patterns (round-nearest-even).

    Much faster than ml_dtypes casting for 0.5GB inputs; inputs are finite
    randn so no inf/nan handling is needed.
    """
    v = np.ascontiguousarray(x_f32).view(np.uint32)
    r = (v + np.uint32(0x7FFF) + ((v >> np.uint32(16)) & np.uint32(1))) >> np.uint32(16)
    return r.astype(np.uint16)


def _host_prep(query, key, mask, lk):
    """Shared host-side input prep (layout + dtype + scalar folding).

    Returns (kt16 [B, DM, lk] uint16-bf16-bits, qs [B-grouped], oh).
    """
    import ml_dtypes

    B = key.shape[0]
    # --- key: bf16 cast + transpose to [B, DM, lk] + zero masked columns
    kt16 = _round_to_bf16_bits(key).transpose(0, 2, 1)
    kt16 = np.ascontiguousarray(kt16)
    kt16 *= (mask[:, None, :] != 0).astype(np.uint16)
    ktb = kt16.view(ml_dtypes.bfloat16)

    # --- qtilde stationaries: qs[p, (b, cc, m)] = qtilde[b, m, d%64]
    #     where d = cc*128 + p, nonzero only when m == d // 64.
    q = query.reshape(B, H, DK).astype(np.float64)
    qn = np.sqrt((q * q).sum(-1))              # [B, H]
    qt = (q / (qn[:, :, None] * float(lk))).astype(np.float32)  # [B, H, 64]
    qsf = np.zeros((B, NCH, P, H), dtype=np.float32)
    p_idx = np.arange(P)
    for cc in range(NCH):
        h_of_p = 2 * cc + p_idx // DK          # head of global d = cc*128+p
        qsf[:, cc, p_idx, h_of_p] = qt[:, h_of_p, p_idx % DK]
    # oh[p, cc*H + m] = 1 iff m == head of global d = cc*128 + p
    ohf = np.zeros((P, NCH, H), dtype=np.float32)
    for cc in range(NCH):
        h_of_p = 2 * cc + p_idx // DK
        ohf[p_idx, cc, h_of_p] = 1.0
    qs16 = _round_to_bf16_bits(qsf).view(ml_dtypes.bfloat16)
    oh16 = _round_to_bf16_bits(ohf.reshape(P, NCH * H)).view(ml_dtypes.bfloat16)
    return ktb, qs16, oh16


def prep_inputs(query, key, mask, n_cores=N_CORES):
    """Per-core input maps (for run_bass_kernel_spmd / bench harnesses)."""
    B, lk, dm = key.shape
    assert dm == DM
    nb = B // n_cores
    ktb, qs16, oh16 = _host_prep(query, key, mask, lk)
    in_maps = []
    for c in range(n_cores):
        sl = slice(c * nb, (c + 1) * nb)
        qs_c = np.ascontiguousarray(
            qs16[sl].transpose(2, 0, 1, 3).reshape(P, nb * NCH * H)
        )
        in_maps.append(
            {
                "ktb": np.ascontiguousarray(ktb[sl]),
                "qs": qs_c,
                "oh": oh16,
            }
        )
    return in_maps


class _Runner:
    """Cached PJRT executable for one built Bass program.

    Mirrors bass2jax.run_bass_via_pjrt but jits ONCE, and feeds the
    global (unsharded) arrays directly: shard_map splits axis 0 across
    the 8 cores, which is exactly the per-core batch shard.
    """

    def __init__(self, nc, n_cores):
        import jax
        from jax.sharding import Mesh, PartitionSpec
        from jax.experimental.shard_map import shard_map
        from concourse import bass2jax as b2j

        b2j.install_neuronx_cc_hook()
        self.jax = jax
        self.n_cores = n_cores
        part_name = (
            nc.partition_id_tensor.name if nc.partition_id_tensor else None
        )
        in_names, out_names, out_avals, zero_outs = [], [], [], []
        for alloc in nc.m.functions[0].allocations:
            if not isinstance(alloc, mybir.MemoryLocationSet):
                continue
            name = alloc.memorylocations[0].name
            if alloc.kind == "ExternalInput":
                if name != part_name:
                    in_names.append(name)
            elif alloc.kind == "ExternalOutput":
                out_names.append(name)
                shape = tuple(alloc.tensor_shape)
                dtype = mybir.dt.np(alloc.dtype)
                out_avals.append(jax.core.ShapedArray(shape, dtype))
                zero_outs.append(np.zeros(shape, dtype))
        self.in_names, self.out_names = in_names, out_names
        self.out_avals, self.zero_outs = out_avals, zero_outs
        n_params, n_outs = len(in_names), len(out_names)

        bind_in_names = in_names + out_names
        if part_name is not None:
            bind_in_names = bind_in_names + [part_name]

        def _body(*args):
            operands = list(args)
            if part_name is not None:
                operands.append(b2j.partition_id_tensor())
            outs = b2j._bass_exec_p.bind(
                *operands,
                out_avals=tuple(out_avals),
                in_names=tuple(bind_in_names),
                out_names=tuple(out_names),
                lowering_input_output_aliases=(),
                sim_require_finite=True,
                sim_require_nnan=True,
                nc=nc,
            )
            return tuple(outs)

        devices = jax.devices()[:n_cores]
        self.mesh = Mesh(np.asarray(devices), ("core",))
        in_specs = (PartitionSpec("core"),) * (n_params + n_outs)
        out_specs = (PartitionSpec("core"),) * n_outs
        self.fn = jax.jit(
            shard_map(
                _body,
                mesh=self.mesh,
                in_specs=in_specs,
                out_specs=out_specs,
                check_rep=False,
            ),
            donate_argnums=tuple(range(n_params, n_params + n_outs)),
            keep_unused=True,
        )

    def global_args(self, global_ins: dict):
        args = [global_ins[name] for name in self.in_names]
        args += [
            np.zeros((self.n_cores * z.shape[0], *z.shape[1:]), z.dtype)
            for z in self.zero_outs
        ]
        return args

    def __call__(self, global_ins: dict):
        out_arrs = self.fn(*self.global_args(global_ins))
        return {
            name: np.asarray(out_arrs[i]) for i, name in enumerate(self.out_names)
        }


_RUNNER_CACHE: dict = {}


def _get_runner(n_batch, lk):
    key = (n_batch, lk)
    if key not in _RUNNER_CACHE:
        nc = _get_nc(n_batch, lk)
        if not nc.is_finalized():
            nc.finalize()
        _RUNNER_CACHE[key] = _Runner(nc, N_CORES)
    return _RUNNER_CACHE[key]


def global_inputs(query, key, mask):
    """Host prep producing the UNSHARDED arrays fed to shard_map (axis 0
    splits evenly across the 8 cores == batch sharding)."""
    B, lk, dm = key.shape
    assert dm == DM
    nb = B // N_CORES
    ktb, qs16, oh16 = _host_prep(query, key, mask, lk)
    # global qs: [N_CORES*128, nb*8*16]; core c reads rows [c*128,(c+1)*128)
    qs_g = np.ascontiguousarray(
        qs16.reshape(N_CORES, nb, NCH, P, H)
        .transpose(0, 3, 1, 2, 4)
        .reshape(N_CORES * P, nb * NCH * H)
    )
    oh_g = np.tile(oh16, (N_CORES, 1))
    return {"ktb": ktb, "qs": qs_g, "oh": oh_g}


def kernel(query, key, mask, trace=False):
    B, lk, _ = key.shape
    nb = B // N_CORES
    runner = _get_runner(nb, lk)
    gins = global_inputs(query, key, mask)
    out = runner(gins)["out"]  # [B, H, lk] concat over cores on axis 0
    full = out.reshape(B, H, lk)
    return full


if __name__ == "__main__":
    # smoke test at reduced size
    rng = np.random.default_rng(0)
    B, lk = 16, 1024
    query = rng.standard_normal((B, 1, DM), dtype=np.float32)
    key = rng.standard_normal((B, lk, DM), dtype=np.float32)
    mask = rng.integers(0, 2, (B, lk)).astype(np.int32)
    out = kernel(query, key, mask)
    print("out", out.shape, out.dtype, float(np.abs(out).max()))

    q = query.reshape(B, H, DK).astype(np.float64)
    k3 = key.reshape(B, lk, H, DK).astype(np.float64)
    num = np.einsum("bhd,blhd->bhl", q, k3)
    qn = np.linalg.norm(q, axis=-1)
    kn = np.linalg.norm(k3, axis=-1).transpose(0, 2, 1)
    scores = num / np.maximum(qn[:, :, None] * kn, 1e-8)
    scores = np.where(mask[:, None, :] == 0, -1e9, scores)
    expected = (np.maximum(scores, 0.0) / lk).astype(np.float32)
    err = np.abs(out - expected).max() / np.abs(expected).max()
    mz = np.all(out[np.broadcast_to(mask[:, None, :] == 0, out.shape)] == 0.0)
    print(f"rel err: {err:.3e}  masked zeros exact: {mz}")


# revision 64
# speedup vs baseline: 1.0502x; 1.0502x over previous
"""Trainium2 Bass kernel for masked cosine-similarity attention scores.

Problem: nn_MultiHeadedAttention_2 (sparse_attention, memory-bound)
  query [16, 1, 1024] f32, key [16, 8192, 1024] f32, mask [16, 8192] int32
  out   [16, 16, 8192] f32 = relu(cos_sim_per_head(q, k) masked) / Lk

Math (per batch b, head h, key position l):
  num[h,l] = sum_d q[h,d] * k[l, h*64+d]
  s2[h,l]  = sum_d k[l, h*64+d]^2          (= kn^2)
  p        = relu(num_tilde) * exp(-0.5*ln(s2 + 1e-8))
  where num_tilde folds q/(qn*Lk) into the weights, and the host zeroes
  the masked key columns so num==0 and s2==0 there: relu(0)*exp(9.2)=0
  gives exact masked zeros. The reference's EPS=1e-8 guard on qn*kn is
  unreachable for randn inputs, so it is not emulated.

Device algorithm (v2 - TensorEngine reduction):
  The host supplies the key tensor cast to bf16 and TRANSPOSED to
  [B, d_model, Lk] with masked columns zeroed. On-chip, d_model lives on
  the partition axis (8 chunks of 128), so both per-head segmented sums
  become PE matmuls contracting over partitions:
    num[16, 512] = sum_cc qs_cc[128,16].T @ kt_cc[128,512]     (8 MMs)
    s2 [16, 512] = sum_cc oh_cc[128,16].T @ Square(kt)_cc      (8 MMs)
  qs_cc holds qtilde = q/(qn*Lk) scattered per chunk (zero off-head),
  oh_cc is the 0/1 head-membership indicator. ACT does Square + Ln/Exp,
  DVE does one relu*mult STT per 512-key tile, GPSIMD only issues the
  small output stores. This removes the v1 DVE/GPSIMD fold trees that
  dominated the measured 434us span (DVE 86% busy).

Sharding: data-parallel over batch B=16 -> 2 batches per core x 8 cores.

Self-contained: only imports the platform libs from /opt/trn_rl_repo.
"""

import sys

sys.path.insert(0, "/opt/trn_rl_repo")

import numpy as np

import concourse.bass as bass
import concourse.mybir as mybir
from concourse.tile import TileContext

# Keep the number of active DMA completion-sem lanes low: the kernel-tail
# Drain waits on every active proc's semaphore and walrus rejects
# instructions with too many sync waits.
import concourse.tile_sem_assignment as _tsa

_tsa.NUM_HWDGE_SEMS = 2
_tsa.NUM_SWDGE_GLOBAL_SEMS = 2

# The walrus build in this environment accepts at most ONE sync wait per
# instruction. Tile's scheduler can emit several (cross-engine RAW + WAR +
# DMA-lane waits). Splitting the extra waits into standalone EventSemaphore
# instructions on the same engine is semantically identical: the engine's
# sequencer executes them in order immediately before the instruction.
import orjson as _orjson


def _split_multi_waits(bir_bytes: bytes) -> bytes:
    m = _orjson.loads(bir_bytes)
    changed = False
    for fn in m.get("functions", []):
        for bb in fn.get("blocks", []):
            insts = bb.get("instructions")
            if not insts:
                continue
            out_list = []
            for inst in insts:
                si = inst.get("sync_info")
                waits = (si or {}).get("on_wait") or []
                if len(waits) > 1:
                    changed = True
                    for k, w in enumerate(waits[:-1]):
                        out_list.append(
                            {
                                "debug": inst.get("debug", 0),
                                "engine": inst["engine"],
                                "ins": [],
                                "name": f"{inst['name']}_wsplit{k}",
                                "opcode": "EventSemaphore",
                                "outs": [],
                                "sync_info": {"on_update": [], "on_wait": [w]},
                            }
                        )
                    si["on_wait"] = [waits[-1]]
                out_list.append(inst)
            bb["instructions"] = out_list
    return _orjson.dumps(m) if changed else bir_bytes


_orig_to_json_bytes = bass.Bass.to_json_bytes


def _patched_to_json_bytes(self, *a, **kw):
    return _split_multi_waits(_orig_to_json_bytes(self, *a, **kw))


bass.Bass.to_json_bytes = _patched_to_json_bytes

F32 = mybir.dt.float32
BF16 = mybir.dt.bfloat16
I32 = mybir.dt.int32
Alu = mybir.AluOpType
Act = mybir.ActivationFunctionType
AX = mybir.AxisListType

H = 16      # heads
DK = 64     # head dim
DM = 1024   # d_model
NCH = DM // 128  # d-chunks of 128 partitions
P = 128     # SBUF partitions
N_CORES = 8
NMM = 512   # matmul moving free size (= PSUM bank capacity in f32)
EPS_S2 = 1e-8


def _block_plan(n_batch: int, lk: int):
    """Per-batch (start, size) block lists: 512-key blocks at the very
    start (short pipeline ramp) and very end (short drain tail), 1024 in
    steady state (fine granularity pipelines best with double-buffered
    num-psum and deep kt prefetch)."""
    plans = []
    for b in range(n_batch):
        if lk <= 1024:
            sizes = [512] * (lk // 512) or [lk]
        else:
            n_big = (lk - 2048) // 2048
            if b == 0:
                sizes = [512, 512, 1024] + [2048] * n_big
            elif b == n_batch - 1:
                sizes = [2048] * n_big + [1024, 512, 512]
            else:
                sizes = [2048] * (n_big + 1)
        blocks, s = [], 0
        for sz in sizes:
            blocks.append((s, sz))
            s += sz
        assert s == lk, (s, lk, sizes)
        plans.append(blocks)
    return plans


def build_nc(n_batch: int, lk: int) -> bass.Bass:
    """Build the per-core Bass program.

    Per-core DRAM I/O:
      ktb [n_batch, 1024, lk] bf16  (transposed, masked-zeroed key shard)
      qs  [128, n_batch*8*16] bf16  (per-chunk qtilde stationaries)
      ohd [128, 4*2*16] fp8e4       (chunk-pair head one-hots, DoubleRow)
      out [n_batch, 16, lk] f32
    """
    plans = _block_plan(n_batch, lk)

    nc = bass.Bass()
    ktb_in = nc.declare_dram_parameter("ktb", [n_batch, DM, lk], BF16,
                                       isOutput=False)
    qs_in = nc.declare_dram_parameter("qs", [P, n_batch * NCH * H], BF16,
                                      isOutput=False)
    oh_in = nc.declare_dram_parameter("oh", [P, NCH * H], BF16,
                                      isOutput=False)
    out = nc.declare_dram_parameter("out", [n_batch, H, lk], F32,
                                    isOutput=True)

    with TileContext(nc) as tc:
        with (
            tc.tile_pool(name="const", bufs=1) as cpool,
            tc.tile_pool(name="kbig", bufs=3) as kpool,
            tc.tile_pool(name="sqp", bufs=2) as qpool,
            tc.tile_pool(name="small", bufs=4) as spool,
            tc.tile_pool(name="outp", bufs=2) as opool,
            tc.tile_pool(name="psumn", bufs=2, space="PSUM") as pnpool,
            tc.tile_pool(name="psums", bufs=3, space="PSUM") as pspool,
        ):
            # consts go on the ACT HWDGE ring (idle at start; carries only
            # the small deferred output stores later) so both key-load rings
            # start streaming immediately
            qs = cpool.tile([P, n_batch * NCH * H], BF16, name="qs")
            nc.scalar.dma_start(out=qs[:], in_=qs_in[:])
            oh = cpool.tile([P, NCH * H], BF16, name="oh")
            nc.scalar.dma_start(out=oh[:], in_=oh_in[:])
            eps = cpool.tile([H, 1], F32, name="eps")
            nc.gpsimd.memset(eps[:], EPS_S2)

            NH = NCH // 2
            pending_out = None
            for b in range(n_batch):
                ktb_b = ktb_in[b].rearrange("(cc p) l -> p cc l", p=P)
                for start, KB in plans[b]:
                    MMW = NMM  # matmul output must stay within one PSUM bank
                    nhalf = KB // MMW
                    # two half-tiles (chunks 0-3 / 4-7), one per load ring:
                    # two transfers concurrently in flight on independent
                    # rings (SP HWDGE / gpsimd SWDGE) measured fastest
                    ktA = kpool.tile([P, NH * KB], BF16, name="ktA", tag="ktA")
                    ktB = kpool.tile([P, NH * KB], BF16, name="ktB", tag="ktB")
                    ktA3 = ktA.rearrange("p (cc l) -> p cc l", l=KB)
                    ktB3 = ktB.rearrange("p (cc l) -> p cc l", l=KB)
                    nc.sync.dma_start(
                        out=ktA3, in_=ktb_b[:, 0:NH, start : start + KB]
                    )
                    nc.gpsimd.dma_start(
                        out=ktB3, in_=ktb_b[:, NH:NCH, start : start + KB]
                    )

                    def kt_cc(cc):
                        return ktA3[:, cc] if cc < NH else ktB3[:, cc - NH]

                    if pending_out is not None:
                        ppout, pb, pstart, pkb = pending_out
                        nc.scalar.dma_start(
                            out=out[pb][:, pstart : pstart + pkb], in_=ppout
                        )
                        pending_out = None

                    # k^2 split across DVE(6)/ACT(2) to balance engine load
                    sqA = qpool.tile([P, NH * KB], BF16, name="sqA", tag="sqA")
                    sqB = qpool.tile([P, NH * KB], BF16, name="sqB", tag="sqB")
                    sqA3 = sqA.rearrange("p (cc l) -> p cc l", l=KB)
                    sqB3 = sqB.rearrange("p (cc l) -> p cc l", l=KB)

                    def sq_cc(cc):
                        return sqA3[:, cc] if cc < NH else sqB3[:, cc - NH]

                    for cc in range(NCH):
                        if cc < 6:
                            nc.vector.tensor_tensor(
                                sq_cc(cc), kt_cc(cc), kt_cc(cc), Alu.mult
                            )
                        else:
                            nc.scalar.activation(sq_cc(cc), kt_cc(cc),
                                                 Act.Square)

                    pout = opool.tile([H, KB], F32, name="pout", tag="pout")
                    # stagger psum partition bases (0 / 64 for the two num
                    # halves, 32 for s2): the matmul col-group follows the
                    # output partition base, so stationaries land in
                    # different PE col-groups and their LDWEIGHTS can be
                    # pulled ahead of in-flight matmuls of the other group
                    pnums = []
                    for h in range(nhalf):
                        off = 64 * (h % 2)
                        t = pnpool.tile([off + H, MMW], F32, name="pnum",
                                        tag=f"pnum{h % 2}")
                        pnums.append(t[off : off + H])
                    # cc-outer: one LDWEIGHTS per chunk feeds all key halves
                    for cc in range(NCH):
                        for h in range(nhalf):
                            nc.tensor.matmul(
                                pnums[h][:],
                                qs[:, (b * NCH + cc) * H : (b * NCH + cc + 1) * H],
                                kt_cc(cc)[:, h * MMW : (h + 1) * MMW],
                                start=(cc == 0),
                                stop=(cc == NCH - 1),
                            )
                    for h in range(nhalf):
                        ksl = slice(h * MMW, (h + 1) * MMW)
                        ps2t = pspool.tile([32 + H, MMW], F32, name="ps2",
                                           tag="ps2")
                        ps2 = ps2t[32 : 32 + H]
                        for cc in range(NCH):
                            nc.tensor.matmul(
                                ps2[:],
                                oh[:, cc * H : (cc + 1) * H],
                                sq_cc(cc)[:, ksl],
                                start=(cc == 0),
                                stop=(cc == NCH - 1),
                            )
                        lns = spool.tile([H, MMW], F32, name="lns", tag="lns")
                        nc.scalar.activation(lns[:], ps2[:], Act.Ln,
                                             bias=eps[:])
                        rk = spool.tile([H, MMW], F32, name="rk", tag="rk")
                        nc.scalar.activation(rk[:], lns[:], Act.Exp,
                                             scale=-0.5)
                        # pout[:, ksl] = max(pnum, 0) * rk
                        nc.vector.scalar_tensor_tensor(
                            pout[:, ksl], pnums[h][:], 0.0, rk[:],
                            Alu.max, Alu.mult,
                        )
                    pending_out = (pout[:], b, start, KB)
            ppout, pb, pstart, pkb = pending_out
            nc.scalar.dma_start(out=out[pb][:, pstart : pstart + pkb],
                                in_=ppout)
    return nc


_NC_CACHE: dict = {}


def _get_nc(n_batch, lk):
    key = (n_batch, lk)
    if key not in _NC_CACHE:
        _NC_CACHE[key] = build_nc(n_batch, lk)
    return _NC_CACHE[key]


def _round_to_bf16_bits(x_f32: np.ndarray) -> np.ndarray:
    """f32 ndarray -> uint16 ndarray of bf16 bit patterns (round-nearest-even).

    Much faster than ml_dtypes casting for 0.5GB inputs; inputs are finite
    randn so no inf/nan handling is needed.
    """
    v = np.ascontiguousarray(x_f32).view(np.uint32)
    r = (v + np.uint32(0x7FFF) + ((v >> np.uint32(16)) & np.uint32(1))) >> np.uint32(16)
    return r.astype(np.uint16)


def _host_prep(query, key, mask, lk):
    """Shared host-side input prep (layout + dtype + scalar folding).

    Returns (kt16 [B, DM, lk] uint16-bf16-bits, qs [B-grouped], oh).
    """
    import ml_dtypes

    B = key.shape[0]
    # --- key: bf16 cast + transpose to [B, DM, lk] + zero masked columns
    kt16 = _round_to_bf16_bits(key).transpose(0, 2, 1)
    kt16 = np.ascontiguousarray(kt16)
    kt16 *= (mask[:, None, :] != 0).astype(np.uint16)
    ktb = kt16.view(ml_dtypes.bfloat16)

    # --- qtilde stationaries: qs[p, (b, cc, m)] = qtilde[b, m, d%64]
    #     where d = cc*128 + p, nonzero only when m == d // 64.
    q = query.reshape(B, H, DK).astype(np.float64)
    qn = np.sqrt((q * q).sum(-1))              # [B, H]
    qt = (q / (qn[:, :, None] * float(lk))).astype(np.float32)  # [B, H, 64]
    qsf = np.zeros((B, NCH, P, H), dtype=np.float32)
    p_idx = np.arange(P)
    for cc in range(NCH):
        h_of_p = 2 * cc + p_idx // DK          # head of global d = cc*128+p
        qsf[:, cc, p_idx, h_of_p] = qt[:, h_of_p, p_idx % DK]
    # oh[p, cc*H + m] = 1 iff m == head of global d = cc*128 + p
    ohf = np.zeros((P, NCH, H), dtype=np.float32)
    for cc in range(NCH):
        h_of_p = 2 * cc + p_idx // DK
        ohf[p_idx, cc, h_of_p] = 1.0
    qs16 = _round_to_bf16_bits(qsf).view(ml_dtypes.bfloat16)
    oh16 = _round_to_bf16_bits(ohf.reshape(P, NCH * H)).view(ml_dtypes.bfloat16)
    return ktb, qs16, oh16


def prep_inputs(query, key, mask, n_cores=N_CORES):
    """Per-core input maps (for run_bass_kernel_spmd / bench harnesses)."""
    B, lk, dm = key.shape
    assert dm == DM
    nb = B // n_cores
    ktb, qs16, oh16 = _host_prep(query, key, mask, lk)
    in_maps = []
    for c in range(n_cores):
        sl = slice(c * nb, (c + 1) * nb)
        qs_c = np.ascontiguousarray(
            qs16[sl].transpose(2, 0, 1, 3).reshape(P, nb * NCH * H)
        )
        in_maps.append(
            {
                "ktb": np.ascontiguousarray(ktb[sl]),
                "qs": qs_c,
                "oh": oh16,
            }
        )
    return in_maps


class _Runner:
    """Cached PJRT executable for one built Bass program.

    Mirrors bass2jax.run_bass_via_pjrt but jits ONCE, and feeds the
    global (unsharded) arrays directly: shard_map splits axis 0 across
    the 8 cores, which is exactly the per-core batch shard.
    """

    def __init__(self, nc, n_cores):
        import jax
        from jax.sharding import Mesh, PartitionSpec
        from jax.experimental.shard_map import shard_map
        from concourse import bass2jax as b2j

        b2j.install_neuronx_cc_hook()
        self.jax = jax
        self.n_cores = n_cores
        part_name = (
            nc.partition_id_tensor.name if nc.partition_id_tensor else None
        )
        in_names, out_names, out_avals, zero_outs = [], [], [], []
        for alloc in nc.m.functions[0].allocations:
            if not isinstance(alloc, mybir.MemoryLocationSet):
                continue
            name = alloc.memorylocations[0].name
            if alloc.kind == "ExternalInput":
                if name != part_name:
                    in_names.append(name)
            elif alloc.kind == "ExternalOutput":
                out_names.append(name)
                shape = tuple(alloc.tensor_shape)
                dtype = mybir.dt.np(alloc.dtype)
                out_avals.append(jax.core.ShapedArray(shape, dtype))
                zero_outs.append(np.zeros(shape, dtype))
        self.in_names, self.out_names = in_names, out_names
        self.out_avals, self.zero_outs = out_avals, zero_outs
        n_params, n_outs = len(in_names), len(out_names)

        bind_in_names = in_names + out_names
        if part_name is not None:
            bind_in_names = bind_in_names + [part_name]

        def _body(*args):
            operands = list(args)
            if part_name is not None:
                operands.append(b2j.partition_id_tensor())
            outs = b2j._bass_exec_p.bind(
                *operands,
                out_avals=tuple(out_avals),
                in_names=tuple(bind_in_names),
                out_names=tuple(out_names),
                lowering_input_output_aliases=(),
                sim_require_finite=True,
                sim_require_nnan=True,
                nc=nc,
            )
            return tuple(outs)

        devices = jax.devices()[:n_cores]
        self.mesh = Mesh(np.asarray(devices), ("core",))
        in_specs = (PartitionSpec("core"),) * (n_params + n_outs)
        out_specs = (PartitionSpec("core"),) * n_outs
        self.fn = jax.jit(
            shard_map(
                _body,
                mesh=self.mesh,
                in_specs=in_specs,
                out_specs=out_specs,
                check_rep=False,
            ),
            donate_argnums=tuple(range(n_params, n_params + n_outs)),
            keep_unused=True,
        )

    def global_args(self, global_ins: dict):
        args = [global_ins[name] for name in self.in_names]
        args += [
            np.zeros((self.n_cores * z.shape[0], *z.shape[1:]), z.dtype)
            for z in self.zero_outs
        ]
        return args

    def __call__(self, global_ins: dict):
        out_arrs = self.fn(*self.global_args(global_ins))
        return {
            name: np.asarray(out_arrs[i]) for i, name in enumerate(self.out_names)
        }


_RUNNER_CACHE: dict = {}


def _get_runner(n_batch, lk):
    key = (n_batch, lk)
    if key not in _RUNNER_CACHE:
        nc = _get_nc(n_batch, lk)
        if not nc.is_finalized():
            nc.finalize()
        _RUNNER_CACHE[key] = _Runner(nc, N_CORES)
    return _RUNNER_CACHE[key]


def global_inputs(query, key, mask):
    """Host prep producing the UNSHARDED arrays fed to shard_map (axis 0
    splits evenly across the 8 cores == batch sharding)."""
    B, lk, dm = key.shape
    assert dm == DM
    nb = B // N_CORES
    ktb, qs16, oh16 = _host_prep(query, key, mask, lk)
    # global qs: [N_CORES*128, nb*8*16]; core c reads rows [c*128,(c+1)*128)
    qs_g = np.ascontiguousarray(
        qs16.reshape(N_CORES, nb, NCH, P, H)
        .transpose(0, 3, 1, 2, 4)
        .reshape(N_CORES * P, nb * NCH * H)
    )
    oh_g = np.tile(oh16, (N_CORES, 1))
    return {"ktb": ktb, "qs": qs_g, "oh": oh_g}


def kernel(query, key, mask, trace=False):
    B, lk, _ = key.shape
    nb = B // N_CORES
    runner = _get_runner(nb, lk)
    gins = global_inputs(query, key, mask)
    out = runner(gins)["out"]  # [B, H, lk] concat over cores on axis 0
    full = out.reshape(B, H, lk)
    return full


if __name__ == "__main__":
    # smoke test at reduced size
    rng = np.random.default_rng(0)
    B, lk = 16, 1024
    query = rng.standard_normal((B, 1, DM), dtype=np.float32)
    key = rng.standard_normal((B, lk, DM), dtype=np.float32)
    mask = rng.integers(0, 2, (B, lk)).astype(np.int32)
    out = kernel(query, key, mask)
    print("out", out.shape, out.dtype, float(np.abs(out).max()))

    q = query.reshape(B, H, DK).astype(np.float64)
    k3 = key.reshape(B, lk, H, DK).astype(np.float64)
    num = np.einsum("bhd,blhd->bhl", q, k3)
    qn = np.linalg.norm(q, axis=-1)
    kn = np.linalg.norm(k3, axis=-1).transpose(0, 2, 1)
    scores = num / np.maximum(qn[:, :, None] * kn, 1e-8)
    scores = np.where(mask[:, None, :] == 0, -1e9, scores)
    expected = (np.maximum(scores, 0.0) / lk).astype(np.float32)
    err = np.abs(out - expected).max() / np.abs(expected).max()
    mz = np.all(out[np.broadcast_to(mask[:, None, :] == 0, out.shape)] == 0.0)
    print(f"rel err: {err:.3e}  masked zeros exact: {mz}")
